# revision 2
# baseline (speedup 1.0000x reference)
"""GCN block (GCNConv + BN(eval) + ReLU) on 8 Trainium2 NeuronCores, v2.

Strategy (fully data-parallel, no collectives, no device gather):
  out = relu(BN(D^{-1/2}(A+I)D^{-1/2} (x W) + b))
      = relu(dis_dst * ((sum_{e->dst} xs[src] + xs[dst]) @ W') + b')
  where xs = x * dis (dis = deg^{-1/2}), W' = W * s, b' = b*s + t (BN folded).

  Nodes are sharded across 8 cores by destination block (degree-balanced
  snake deal).  The HOST pre-expands the per-edge source rows into a
  contiguous [128, CT, F] bf16 array per core (slot-major groups of 128
  edge slots per dst tile; the first group of each tile holds the
  self-loop rows).  The device then:
    - bulk-DMAs edge-row groups (HWDGE, line-rate; no gpsimd gather),
    - builds one-hot selection matrices on-chip with DVE is_equal
      (iota vs per-slot dst labels),
    - aggregates with sel-stationary matmuls: agg[dst,f] += sel^T @ G
      (one N=512 matmul per 128-slot group),
    - transposes agg via 4 identity matmuls, then the 512x512 transform
      GEMM + K=1 bias matmul (bias pre-scaled by 1/dis so the final
      per-partition dis scale fuses into the ReLU activation),
    - writes bf16 output (host casts to f32).
"""

import sys

if "/opt/trn_rl_repo" not in sys.path:
    sys.path.insert(0, "/opt/trn_rl_repo")

import math

import ml_dtypes
import numpy as np

BF16 = ml_dtypes.bfloat16

N_CORES = 8
P = 128
BN_EPS = 1e-5


def _prep(x, edge_index, W, b, gamma, beta, running_mean, running_var):
    """Host-side preprocessing: sharding, edge-row expansion, BN folding."""
    N, F = x.shape
    F_OUT = W.shape[1]
    KC = F // P
    assert N % N_CORES == 0
    NB = N // N_CORES
    T = math.ceil(NB / P)  # dst tiles per core

    src = np.asarray(edge_index[0], dtype=np.int64)
    dst = np.asarray(edge_index[1], dtype=np.int64)

    deg = 1.0 + np.bincount(dst, minlength=N).astype(np.float64)
    dis = (1.0 / np.sqrt(deg)).astype(np.float32)

    xs = (np.asarray(x, np.float32) * dis[:, None]).astype(BF16)

    # BN folding
    s = (np.asarray(gamma, np.float32)
         / np.sqrt(np.asarray(running_var, np.float32) + BN_EPS))
    t = np.asarray(beta, np.float32) - np.asarray(running_mean, np.float32) * s
    Wp = (np.asarray(W, np.float32) * s[None, :]).astype(BF16)
    bp = (np.asarray(b, np.float32) * s + t).astype(BF16)
    wp = np.ascontiguousarray(Wp.reshape(KC, P, F_OUT).transpose(1, 0, 2))

    # ---- degree-balanced node -> (core, tile, slot) assignment (snake deal)
    NBINS = N_CORES * T
    order = np.argsort(-(deg - 1.0), kind="stable")
    assign = np.empty(N, np.int64)   # node -> bin
    slot_of = np.empty(N, np.int64)  # node -> slot within bin
    pos = 0
    rnd = 0
    while pos < N:
        chunk = order[pos:pos + NBINS]
        if rnd % 2 == 0:
            bins = np.arange(len(chunk))
        else:
            bins = NBINS - 1 - np.arange(len(chunk))
        assign[chunk] = bins
        slot_of[chunk] = rnd
        pos += NBINS
        rnd += 1
    assert rnd <= P, f"too many slot rounds {rnd}"
    core_of_bin = assign % N_CORES
    tile_of_bin = assign // N_CORES

    # node_map[k][t, p] = original node id (or -1)
    node_map = np.full((N_CORES, T, P), -1, dtype=np.int64)
    node_map[core_of_bin, tile_of_bin, slot_of] = np.arange(N)

    e_core = core_of_bin[dst]
    e_tile = tile_of_bin[dst]
    e_slot = slot_of[dst]

    # ---- pass 1: per-core edge lists sorted by tile, per-tile counts
    per_core = []
    cnt = np.zeros((N_CORES, T), dtype=np.int64)
    for k in range(N_CORES):
        m = e_core == k
        s_k = src[m]
        t_k = e_tile[m]
        p_k = e_slot[m]
        o = np.argsort(t_k, kind="stable")
        s_k, t_k, p_k = s_k[o], t_k[o], p_k[o]
        bounds = np.searchsorted(t_k, np.arange(T + 1))
        cnt[k] = bounds[1:] - bounds[:-1]
        per_core.append((s_k, p_k, bounds))

    # slots per tile: 128 self slots + edges, padded to 128 (uniform over cores)
    NG_t = (1 + np.ceil(cnt.max(axis=0) / P).astype(np.int64))
    goff_t = np.concatenate([[0], np.cumsum(NG_t)])
    CT = int(goff_t[-1])  # total 128-slot column groups per core

    # ---- pass 2: per-core expanded arrays
    in_maps = []
    for k in range(N_CORES):
        s_k, p_k, bounds = per_core[k]
        srcidx = np.zeros((CT, P), dtype=np.int64)    # slot -> source row
        dstl = np.full((CT, P), -1.0, dtype=np.float32)  # slot -> dst slot
        nm = node_map[k]  # [T, P]
        valid = nm >= 0
        for t in range(T):
            g0 = goff_t[t]
            # group 0: self-loop rows (slot order == dst order -> sel = I)
            srcidx[g0] = np.where(valid[t], nm[t], 0)
            dstl[g0] = np.where(valid[t], np.arange(P, dtype=np.float32), -1.0)
            e_lo, e_hi = bounds[t], bounds[t + 1]
            n_e = e_hi - e_lo
            flat_lo = (g0 + 1) * P
            sflat = srcidx.reshape(-1)
            dflat = dstl.reshape(-1)
            sflat[flat_lo:flat_lo + n_e] = s_k[e_lo:e_hi]
            dflat[flat_lo:flat_lo + n_e] = p_k[e_lo:e_hi].astype(np.float32)
        # gexp[s, c, :] = xs[srcidx[c, s]]
        gexp = xs[srcidx.T]  # [128, CT, F] bf16
        dstl_pk = np.ascontiguousarray(dstl.T).astype(np.float32)  # [128, CT]

        iota = np.ascontiguousarray(np.broadcast_to(
            np.arange(P, dtype=np.float32), (P, P)).astype(BF16))
        ident = np.eye(P, dtype=np.float32).astype(BF16)

        nm_safe = np.where(valid, nm, 0)
        dis_tp = np.where(valid, dis[nm_safe], 1.0).astype(np.float32)  # [T, P]
        dis_t = np.ascontiguousarray(dis_tp.T)  # [128, T]
        invdis = np.zeros((1, T * P), dtype=BF16)
        invdis[0, :] = np.where(valid, 1.0 / np.maximum(dis_tp, 1e-9), 0.0
                                ).reshape(-1).astype(BF16)
        in_maps.append({
            "gexp": np.ascontiguousarray(gexp),
            "dstl": dstl_pk,
            "iota": iota,
            "ident": np.ascontiguousarray(ident),
            "dis_t": dis_t,
            "invdis": invdis,
            "wp": wp,
            "bp": bp.reshape(1, F_OUT),
        })

    meta = {
        "N": N, "F": F, "F_OUT": F_OUT, "KC": KC, "NB": NB, "T": T,
        "CT": CT, "NG_t": NG_t.tolist(), "goff_t": goff_t.tolist(),
        "node_map": node_map,
    }
    return meta, in_maps


def _build_program(meta):
    """Emit the Bass/Tile program (shared by all cores)."""
    import concourse.bacc as bacc
    import concourse.mybir as mybir
    import concourse.tile as tile

    F, F_OUT, KC = meta["F"], meta["F_OUT"], meta["KC"]
    T, CT = meta["T"], meta["CT"]
    NG_t, goff_t = meta["NG_t"], meta["goff_t"]
    NGMAX = max(NG_t)
    TB = 8  # output write batch (tiles)

    dt = mybir.dt
    nc = bacc.Bacc("TRN2", target_bir_lowering=False, debug=False,
                   enable_asserts=False, num_devices=N_CORES)

    gexp = nc.dram_tensor("gexp", [P, CT, F], dt.bfloat16, kind="ExternalInput").ap()
    dstl = nc.dram_tensor("dstl", [P, CT], dt.float32, kind="ExternalInput").ap()
    iota = nc.dram_tensor("iota", [P, P], dt.bfloat16, kind="ExternalInput").ap()
    ident = nc.dram_tensor("ident", [P, P], dt.bfloat16, kind="ExternalInput").ap()
    dis_t = nc.dram_tensor("dis_t", [P, T], dt.float32, kind="ExternalInput").ap()
    invdis = nc.dram_tensor("invdis", [1, T * P], dt.bfloat16, kind="ExternalInput").ap()
    wp = nc.dram_tensor("wp", [P, KC, F_OUT], dt.bfloat16, kind="ExternalInput").ap()
    bp = nc.dram_tensor("bp", [1, F_OUT], dt.bfloat16, kind="ExternalInput").ap()
    out = nc.dram_tensor("out", [P, T, F_OUT], dt.bfloat16, kind="ExternalOutput").ap()

    with tile.TileContext(nc) as tc:
        with (
            tc.tile_pool(name="const", bufs=1) as cpool,
            tc.tile_pool(name="gbuf", bufs=12) as gpool,
            tc.tile_pool(name="sel", bufs=6) as selpool,
            tc.tile_pool(name="agg", bufs=3) as aggpool,
            tc.tile_pool(name="aggT", bufs=3) as aggTpool,
            tc.tile_pool(name="outsb", bufs=3) as opool,
            tc.tile_pool(name="psA", bufs=3, space="PSUM") as psA,
            tc.tile_pool(name="psB", bufs=3, space="PSUM") as psB,
            tc.tile_pool(name="psC", bufs=2, space="PSUM") as psC,
        ):
            # resident constants; sel-build inputs (iota, dstl) first so tile 0
            # work can start ASAP, heavyweight transform consts via the
            # scalar-engine HWDGE ring so they don't block gexp loads.
            iota_sb = cpool.tile([P, P], dt.bfloat16, tag="iota")
            nc.sync.dma_start(iota_sb[:], iota[:])
            dstl_sb = cpool.tile([P, CT], dt.float32, tag="dstl")
            nc.sync.dma_start(dstl_sb[:], dstl[:])
            ident_sb = cpool.tile([P, P], dt.bfloat16, tag="ident")
            nc.scalar.dma_start(ident_sb[:], ident[:])
            dis_sb = cpool.tile([P, T], dt.float32, tag="dis")
            nc.scalar.dma_start(dis_sb[:], dis_t[:])
            invdis_sb = cpool.tile([1, T * P], dt.bfloat16, tag="invdis")
            nc.scalar.dma_start(invdis_sb[:], invdis[:])
            wp_sb = cpool.tile([P, KC, F_OUT], dt.bfloat16, tag="wp")
            nc.scalar.dma_start(wp_sb[:], wp[:])
            bp_sb = cpool.tile([1, F_OUT], dt.bfloat16, tag="bp")
            nc.scalar.dma_start(bp_sb[:], bp[:])

            # 3-stage software pipeline over tiles:
            #   iter t emits  agg(t) | transpose(t-1) | transform+relu(t-2)
            # so each PE stage's PSUM->SBUF copy has a full agg-phase of
            # slack before the consuming matmuls are issued.
            state = {}  # tile -> dict of live tiles
            out_blk = [None]

            def stage_agg(t):
                ng = NG_t[t]
                g0 = goff_t[t]
                g_sb = gpool.tile([P, NGMAX, F], dt.bfloat16, tag="g")
                nc.sync.dma_start(g_sb[:, :ng, :], gexp[:, g0:g0 + ng, :])
                # on-chip one-hot selection matrices: sel[slot, dst]
                sel_sb = selpool.tile([P, NGMAX * P], dt.bfloat16, tag="sel")
                for g in range(ng):
                    nc.vector.tensor_scalar(
                        out=sel_sb[:, g * P:(g + 1) * P],
                        in0=iota_sb[:],
                        scalar1=dstl_sb[:, g0 + g:g0 + g + 1],
                        scalar2=None,
                        op0=mybir.AluOpType.is_equal)
                # aggregation: agg[dst, f] = sum_g sel_g^T @ G_g
                agg_ps = psA.tile([P, F], dt.float32, tag="agg_ps")
                for g in range(ng):
                    nc.tensor.matmul(
                        agg_ps[:],
                        lhsT=sel_sb[:, g * P:(g + 1) * P],
                        rhs=g_sb[:, g, :],
                        start=(g == 0),
                        stop=(g == ng - 1),
                        skip_group_check=True,
                    )
                agg_sb = aggpool.tile([P, F], dt.bfloat16, tag="agg_sb")
                nc.scalar.activation(
                    agg_sb[:], agg_ps[:],
                    mybir.ActivationFunctionType.Copy)
                state[t] = {"agg_sb": agg_sb}

            def stage_transpose(t):
                st = state[t]
                agg_sb = st["agg_sb"]
                aggT_ps = psB.tile([P, F], dt.float32, tag="aggT_ps")
                for c in range(KC):
                    nc.tensor.matmul(
                        aggT_ps[:, c * P:(c + 1) * P],
                        lhsT=agg_sb[:, c * P:(c + 1) * P],
                        rhs=ident_sb[:],
                        start=(c == 0),
                        stop=(c == KC - 1),
                        skip_group_check=True,
                    )
                aggT_sb = aggTpool.tile([P, F], dt.bfloat16, tag="aggT_sb")
                nc.scalar.activation(
                    aggT_sb[:], aggT_ps[:],
                    mybir.ActivationFunctionType.Copy)
                st["aggT_sb"] = aggT_sb

            def stage_transform(t):
                st = state.pop(t)
                aggT_sb = st["aggT_sb"]
                if t % TB == 0:
                    ob = opool.tile([P, TB, F_OUT], dt.bfloat16, tag="out_sb")
                    out_blk[0] = ob
                out_ps = psC.tile([P, F_OUT], dt.float32, tag="out_ps")
                for c in range(KC):
                    nc.tensor.matmul(
                        out_ps[:],
                        lhsT=aggT_sb[:, c * P:(c + 1) * P],
                        rhs=wp_sb[:, c, :],
                        start=(c == 0),
                        stop=False,
                    )
                nc.tensor.matmul(
                    out_ps[:],
                    lhsT=invdis_sb[:1, t * P:(t + 1) * P],
                    rhs=bp_sb[:1, :],
                    start=False,
                    stop=True,
                )
                nc.scalar.activation(
                    out_blk[0][:, t % TB, :],
                    out_ps[:],
                    mybir.ActivationFunctionType.Relu,
                    scale=dis_sb[:, t:t + 1],
                )
                if t % TB == TB - 1 or t == T - 1:
                    t0 = (t // TB) * TB
                    nc.scalar.dma_start(out[:, t0:t + 1, :],
                                        out_blk[0][:, :t - t0 + 1, :])

            for t in range(T + 2):
                if t < T:
                    stage_agg(t)
                if 1 <= t < T + 1:
                    stage_transpose(t - 1)
                if t >= 2:
                    stage_transform(t - 2)

    nc.compile()
    return nc


_CACHE = {}


def _get_program(meta):
    key = (meta["N"], meta["F"], meta["F_OUT"], meta["CT"],
           tuple(meta["NG_t"]))
    if key not in _CACHE:
        _CACHE[key] = _build_program(meta)
    return _CACHE[key]


def kernel(x, edge_index, W, b, gamma, beta, running_mean, running_var,
           _want_results_holder=None, _run_kwargs=None):
    meta, in_maps = _prep(x, edge_index, W, b, gamma, beta,
                          running_mean, running_var)
    nc = _get_program(meta)

    from concourse.bass_utils import run_bass_kernel_spmd

    res = run_bass_kernel_spmd(nc, in_maps, core_ids=list(range(N_CORES)),
                               **(_run_kwargs or {}))
    if _want_results_holder is not None:
        _want_results_holder.append((nc, meta, in_maps, res))

    T, F_OUT = meta["T"], meta["F_OUT"]
    node_map = meta["node_map"]
    out = np.empty((meta["N"], F_OUT), dtype=np.float32)
    for k in range(N_CORES):
        tiled = np.asarray(res.results[k]["out"], dtype=np.float32)
        rows = np.ascontiguousarray(tiled.transpose(1, 0, 2))  # [T, 128, F]
        nm = node_map[k]
        valid = nm >= 0
        out[nm[valid]] = rows[valid]
    return out


# revision 3
# speedup vs baseline: 1.0923x; 1.0923x over previous
"""GCN block (GCNConv + BN(eval) + ReLU) on 8 Trainium2 NeuronCores, v2.

Strategy (fully data-parallel, no collectives, no device gather):
  out = relu(BN(D^{-1/2}(A+I)D^{-1/2} (x W) + b))
      = relu(dis_dst * ((sum_{e->dst} xs[src] + xs[dst]) @ W') + b')
  where xs = x * dis (dis = deg^{-1/2}), W' = W * s, b' = b*s + t (BN folded).

  Nodes are sharded across 8 cores by destination block (degree-balanced
  snake deal).  The HOST pre-expands the per-edge source rows into a
  contiguous [128, CT, F] bf16 array per core (slot-major groups of 128
  edge slots per dst tile; the first group of each tile holds the
  self-loop rows).  The device then:
    - bulk-DMAs edge-row groups (HWDGE, line-rate; no gpsimd gather),
    - builds one-hot selection matrices on-chip with DVE is_equal
      (iota vs per-slot dst labels),
    - aggregates with sel-stationary matmuls: agg[dst,f] += sel^T @ G
      (one N=512 matmul per 128-slot group),
    - transposes agg via 4 identity matmuls, then the 512x512 transform
      GEMM + K=1 bias matmul (bias pre-scaled by 1/dis so the final
      per-partition dis scale fuses into the ReLU activation),
    - writes bf16 output (host casts to f32).
"""

import sys

if "/opt/trn_rl_repo" not in sys.path:
    sys.path.insert(0, "/opt/trn_rl_repo")

import math

import ml_dtypes
import numpy as np

BF16 = ml_dtypes.bfloat16

N_CORES = 8
P = 128
BN_EPS = 1e-5


def _prep(x, edge_index, W, b, gamma, beta, running_mean, running_var):
    """Host-side preprocessing: sharding, edge-row expansion, BN folding."""
    N, F = x.shape
    F_OUT = W.shape[1]
    KC = F // P
    assert N % N_CORES == 0
    NB = N // N_CORES
    T = math.ceil(NB / P)  # dst tiles per core

    src = np.asarray(edge_index[0], dtype=np.int64)
    dst = np.asarray(edge_index[1], dtype=np.int64)

    deg = 1.0 + np.bincount(dst, minlength=N).astype(np.float64)
    dis = (1.0 / np.sqrt(deg)).astype(np.float32)

    xs = (np.asarray(x, np.float32) * dis[:, None]).astype(BF16)

    # BN folding
    s = (np.asarray(gamma, np.float32)
         / np.sqrt(np.asarray(running_var, np.float32) + BN_EPS))
    t = np.asarray(beta, np.float32) - np.asarray(running_mean, np.float32) * s
    Wp = (np.asarray(W, np.float32) * s[None, :]).astype(BF16)
    bp = (np.asarray(b, np.float32) * s + t).astype(BF16)
    wp = np.ascontiguousarray(Wp.reshape(KC, P, F_OUT).transpose(1, 0, 2))

    # ---- degree-balanced node -> (core, tile, slot) assignment (snake deal)
    NBINS = N_CORES * T
    order = np.argsort(-(deg - 1.0), kind="stable")
    assign = np.empty(N, np.int64)   # node -> bin
    slot_of = np.empty(N, np.int64)  # node -> slot within bin
    pos = 0
    rnd = 0
    while pos < N:
        chunk = order[pos:pos + NBINS]
        if rnd % 2 == 0:
            bins = np.arange(len(chunk))
        else:
            bins = NBINS - 1 - np.arange(len(chunk))
        assign[chunk] = bins
        slot_of[chunk] = rnd
        pos += NBINS
        rnd += 1
    assert rnd <= P, f"too many slot rounds {rnd}"
    core_of_bin = assign % N_CORES
    tile_of_bin = assign // N_CORES

    # node_map[k][t, p] = original node id (or -1)
    node_map = np.full((N_CORES, T, P), -1, dtype=np.int64)
    node_map[core_of_bin, tile_of_bin, slot_of] = np.arange(N)

    e_core = core_of_bin[dst]
    e_tile = tile_of_bin[dst]
    e_slot = slot_of[dst]

    # ---- pass 1: per-core edge lists sorted by tile, per-tile counts
    per_core = []
    cnt = np.zeros((N_CORES, T), dtype=np.int64)
    for k in range(N_CORES):
        m = e_core == k
        s_k = src[m]
        t_k = e_tile[m]
        p_k = e_slot[m]
        o = np.argsort(t_k, kind="stable")
        s_k, t_k, p_k = s_k[o], t_k[o], p_k[o]
        bounds = np.searchsorted(t_k, np.arange(T + 1))
        cnt[k] = bounds[1:] - bounds[:-1]
        per_core.append((s_k, p_k, bounds))

    # slots per tile: 128 self slots + edges, padded to 128 (uniform over cores)
    NG_t = (1 + np.ceil(cnt.max(axis=0) / P).astype(np.int64))
    goff_t = np.concatenate([[0], np.cumsum(NG_t)])
    CT = int(goff_t[-1])  # total 128-slot column groups per core

    # ---- pass 2: per-core expanded arrays
    in_maps = []
    for k in range(N_CORES):
        s_k, p_k, bounds = per_core[k]
        srcidx = np.zeros((CT, P), dtype=np.int64)    # slot -> source row
        dstl = np.full((CT, P), -1.0, dtype=np.float32)  # slot -> dst slot
        nm = node_map[k]  # [T, P]
        valid = nm >= 0
        for t in range(T):
            g0 = goff_t[t]
            # group 0: self-loop rows (slot order == dst order -> sel = I)
            srcidx[g0] = np.where(valid[t], nm[t], 0)
            dstl[g0] = np.where(valid[t], np.arange(P, dtype=np.float32), -1.0)
            e_lo, e_hi = bounds[t], bounds[t + 1]
            n_e = e_hi - e_lo
            flat_lo = (g0 + 1) * P
            sflat = srcidx.reshape(-1)
            dflat = dstl.reshape(-1)
            sflat[flat_lo:flat_lo + n_e] = s_k[e_lo:e_hi]
            dflat[flat_lo:flat_lo + n_e] = p_k[e_lo:e_hi].astype(np.float32)
        # gexp[s, c, :] = xs[srcidx[c, s]]
        gexp = xs[srcidx.T]  # [128, CT, F] bf16
        dstl_pk = np.ascontiguousarray(dstl.T).astype(np.float32)  # [128, CT]

        iota = np.ascontiguousarray(np.broadcast_to(
            np.arange(P, dtype=np.float32), (P, P)).astype(BF16))
        ident = np.eye(P, dtype=np.float32).astype(BF16)

        nm_safe = np.where(valid, nm, 0)
        dis_tp = np.where(valid, dis[nm_safe], 1.0).astype(np.float32)  # [T, P]
        dis_t = np.ascontiguousarray(dis_tp.T)  # [128, T]
        invdis = np.zeros((1, T * P), dtype=BF16)
        invdis[0, :] = np.where(valid, 1.0 / np.maximum(dis_tp, 1e-9), 0.0
                                ).reshape(-1).astype(BF16)
        in_maps.append({
            "gexp": np.ascontiguousarray(gexp),
            "dstl": dstl_pk,
            "iota": iota,
            "ident": np.ascontiguousarray(ident),
            "dis_t": dis_t,
            "invdis": invdis,
            "wp": wp,
            "bp": bp.reshape(1, F_OUT),
        })

    meta = {
        "N": N, "F": F, "F_OUT": F_OUT, "KC": KC, "NB": NB, "T": T,
        "CT": CT, "NG_t": NG_t.tolist(), "goff_t": goff_t.tolist(),
        "node_map": node_map,
    }
    return meta, in_maps


def _build_program(meta):
    """Emit the Bass/Tile program (shared by all cores)."""
    import concourse.bacc as bacc
    import concourse.mybir as mybir
    import concourse.tile as tile

    F, F_OUT, KC = meta["F"], meta["F_OUT"], meta["KC"]
    T, CT = meta["T"], meta["CT"]
    NG_t, goff_t = meta["NG_t"], meta["goff_t"]
    NGMAX = max(NG_t)
    TB = 8  # output write batch (tiles)

    dt = mybir.dt
    nc = bacc.Bacc("TRN2", target_bir_lowering=False, debug=False,
                   enable_asserts=False, num_devices=N_CORES)

    gexp = nc.dram_tensor("gexp", [P, CT, F], dt.bfloat16, kind="ExternalInput").ap()
    dstl = nc.dram_tensor("dstl", [P, CT], dt.float32, kind="ExternalInput").ap()
    iota = nc.dram_tensor("iota", [P, P], dt.bfloat16, kind="ExternalInput").ap()
    ident = nc.dram_tensor("ident", [P, P], dt.bfloat16, kind="ExternalInput").ap()
    dis_t = nc.dram_tensor("dis_t", [P, T], dt.float32, kind="ExternalInput").ap()
    invdis = nc.dram_tensor("invdis", [1, T * P], dt.bfloat16, kind="ExternalInput").ap()
    wp = nc.dram_tensor("wp", [P, KC, F_OUT], dt.bfloat16, kind="ExternalInput").ap()
    bp = nc.dram_tensor("bp", [1, F_OUT], dt.bfloat16, kind="ExternalInput").ap()
    out = nc.dram_tensor("out", [P, T, F_OUT], dt.bfloat16, kind="ExternalOutput").ap()

    with tile.TileContext(nc) as tc:
        with (
            tc.tile_pool(name="const", bufs=1) as cpool,
            tc.tile_pool(name="gbuf", bufs=12) as gpool,
            tc.tile_pool(name="sel", bufs=6) as selpool,
            tc.tile_pool(name="agg", bufs=3) as aggpool,
            tc.tile_pool(name="aggT", bufs=3) as aggTpool,
            tc.tile_pool(name="outsb", bufs=3) as opool,
            tc.tile_pool(name="psA", bufs=3, space="PSUM") as psA,
            tc.tile_pool(name="psB", bufs=3, space="PSUM") as psB,
            tc.tile_pool(name="psC", bufs=2, space="PSUM") as psC,
        ):
            # resident constants; sel-build inputs (iota, dstl) first so tile 0
            # work can start ASAP, heavyweight transform consts via the
            # scalar-engine HWDGE ring so they don't block gexp loads.
            iota_sb = cpool.tile([P, P], dt.bfloat16, tag="iota")
            nc.sync.dma_start(iota_sb[:], iota[:])
            dstl_sb = cpool.tile([P, CT], dt.float32, tag="dstl")
            nc.scalar.dma_start(dstl_sb[:], dstl[:])
            ident_sb = cpool.tile([P, P], dt.bfloat16, tag="ident")
            nc.scalar.dma_start(ident_sb[:], ident[:])
            dis_sb = cpool.tile([P, T], dt.float32, tag="dis")
            nc.scalar.dma_start(dis_sb[:], dis_t[:])
            invdis_sb = cpool.tile([1, T * P], dt.bfloat16, tag="invdis")
            nc.scalar.dma_start(invdis_sb[:], invdis[:])
            wp_sb = cpool.tile([P, KC, F_OUT], dt.bfloat16, tag="wp")
            nc.scalar.dma_start(wp_sb[:], wp[:])
            bp_sb = cpool.tile([1, F_OUT], dt.bfloat16, tag="bp")
            nc.scalar.dma_start(bp_sb[:], bp[:])

            # PE warm-up: ~2us of dummy matmuls during the DMA fill so the
            # HAM clock gate is released before tile 0's real matmuls.
            # (borrows a psC slot; released before the first transform)
            warm_ps = psC.tile([P, F_OUT], dt.float32, tag="out_ps")
            for w in range(20):
                nc.tensor.matmul(
                    warm_ps[:, :P],
                    lhsT=iota_sb[:],
                    rhs=iota_sb[:],
                    start=(w == 0),
                    stop=(w == 19),
                    skip_group_check=True,
                )

            # 3-stage software pipeline over tiles:
            #   iter t emits  agg(t) | transpose(t-1) | transform+relu(t-2)
            # so each PE stage's PSUM->SBUF copy has a full agg-phase of
            # slack before the consuming matmuls are issued.
            state = {}  # tile -> dict of live tiles
            out_blk = [None]

            def stage_agg(t):
                ng = NG_t[t]
                g0 = goff_t[t]
                g_sb = gpool.tile([P, NGMAX, F], dt.bfloat16, tag="g")
                nc.sync.dma_start(g_sb[:, :ng, :], gexp[:, g0:g0 + ng, :])
                # on-chip one-hot selection matrices: sel[slot, dst]
                sel_sb = selpool.tile([P, NGMAX * P], dt.bfloat16, tag="sel")
                for g in range(ng):
                    nc.vector.tensor_scalar(
                        out=sel_sb[:, g * P:(g + 1) * P],
                        in0=iota_sb[:],
                        scalar1=dstl_sb[:, g0 + g:g0 + g + 1],
                        scalar2=None,
                        op0=mybir.AluOpType.is_equal)
                # aggregation: agg[dst, f] = sum_g sel_g^T @ G_g
                agg_ps = psA.tile([P, F], dt.float32, tag="agg_ps")
                for g in range(ng):
                    nc.tensor.matmul(
                        agg_ps[:],
                        lhsT=sel_sb[:, g * P:(g + 1) * P],
                        rhs=g_sb[:, g, :],
                        start=(g == 0),
                        stop=(g == ng - 1),
                        skip_group_check=True,
                    )
                agg_sb = aggpool.tile([P, F], dt.bfloat16, tag="agg_sb")
                nc.scalar.activation(
                    agg_sb[:], agg_ps[:],
                    mybir.ActivationFunctionType.Copy)
                state[t] = {"agg_sb": agg_sb}

            def stage_transpose(t):
                st = state[t]
                agg_sb = st["agg_sb"]
                aggT_ps = psB.tile([P, F], dt.float32, tag="aggT_ps")
                for c in range(KC):
                    nc.tensor.matmul(
                        aggT_ps[:, c * P:(c + 1) * P],
                        lhsT=agg_sb[:, c * P:(c + 1) * P],
                        rhs=ident_sb[:],
                        start=(c == 0),
                        stop=(c == KC - 1),
                        skip_group_check=True,
                    )
                aggT_sb = aggTpool.tile([P, F], dt.bfloat16, tag="aggT_sb")
                nc.scalar.activation(
                    aggT_sb[:], aggT_ps[:],
                    mybir.ActivationFunctionType.Copy)
                st["aggT_sb"] = aggT_sb

            def stage_transform(t):
                st = state.pop(t)
                aggT_sb = st["aggT_sb"]
                if t % TB == 0:
                    ob = opool.tile([P, TB, F_OUT], dt.bfloat16, tag="out_sb")
                    out_blk[0] = ob
                out_ps = psC.tile([P, F_OUT], dt.float32, tag="out_ps")
                for c in range(KC):
                    nc.tensor.matmul(
                        out_ps[:],
                        lhsT=aggT_sb[:, c * P:(c + 1) * P],
                        rhs=wp_sb[:, c, :],
                        start=(c == 0),
                        stop=False,
                    )
                nc.tensor.matmul(
                    out_ps[:],
                    lhsT=invdis_sb[:1, t * P:(t + 1) * P],
                    rhs=bp_sb[:1, :],
                    start=False,
                    stop=True,
                )
                nc.scalar.activation(
                    out_blk[0][:, t % TB, :],
                    out_ps[:],
                    mybir.ActivationFunctionType.Relu,
                    scale=dis_sb[:, t:t + 1],
                )
                if t % TB == TB - 1 or t == T - 1:
                    t0 = (t // TB) * TB
                    nc.scalar.dma_start(out[:, t0:t + 1, :],
                                        out_blk[0][:, :t - t0 + 1, :])

            for t in range(T + 2):
                if t < T:
                    stage_agg(t)
                if 1 <= t < T + 1:
                    stage_transpose(t - 1)
                if t >= 2:
                    stage_transform(t - 2)

    nc.compile()
    return nc


_CACHE = {}


def _get_program(meta):
    key = (meta["N"], meta["F"], meta["F_OUT"], meta["CT"],
           tuple(meta["NG_t"]))
    if key not in _CACHE:
        _CACHE[key] = _build_program(meta)
    return _CACHE[key]


def kernel(x, edge_index, W, b, gamma, beta, running_mean, running_var,
           _want_results_holder=None, _run_kwargs=None):
    meta, in_maps = _prep(x, edge_index, W, b, gamma, beta,
                          running_mean, running_var)
    nc = _get_program(meta)

    from concourse.bass_utils import run_bass_kernel_spmd

    res = run_bass_kernel_spmd(nc, in_maps, core_ids=list(range(N_CORES)),
                               **(_run_kwargs or {}))
    if _want_results_holder is not None:
        _want_results_holder.append((nc, meta, in_maps, res))

    T, F_OUT = meta["T"], meta["F_OUT"]
    node_map = meta["node_map"]
    out = np.empty((meta["N"], F_OUT), dtype=np.float32)
    for k in range(N_CORES):
        tiled = np.asarray(res.results[k]["out"], dtype=np.float32)
        rows = np.ascontiguousarray(tiled.transpose(1, 0, 2))  # [T, 128, F]
        nm = node_map[k]
        valid = nm >= 0
        out[nm[valid]] = rows[valid]
    return out
